# revision 27
# baseline (speedup 1.0000x reference)
"""MiniBatchDiscrimination Trainium2 kernel (8-core SPMD, symmetric pairs).

Reference computation:
    m = (x @ T).reshape(B, OUT_F, NUM_K)            # B=256, OUT_F=128, NUM_K=16
    dists = |m[None,:,:,:] - m[:,None,:,:]|         # [B, B, OUT_F, NUM_K]
    out = sum_i exp(-sum_k dists) - 1               # [B, OUT_F]
    return concat([x, out], axis=-1)                # [B, 640]

Strategy:
  * The pairwise matrix is symmetric.  Partition the 256 rows into 16
    strips of 16; the 136 unordered strip pairs are covered by 8
    edge-disjoint trails of 17 edges (Walecki Hamiltonian paths of K16 plus
    one loop per vertex, two loops inserted per path).  Core c walks trail
    c: 18 strip slots, 17 edges.  Consecutive edge pairs (2d, 2d+1) share
    slot 2d+1; processing edge 2d transposed makes slot 2d+1 the common
    i-side, so each "double unit" d computes one 16i x 32j block
    (i = slot[2d+1], j = slots[2d] ++ slots[2d+2]) with 512-column
    matmuls; edge 16 is a single 16x16 unit.  Every ordered pair is
    computed exactly once globally.
  * GEMM: x/T in fp8e4, DoubleRow matmuls.  m bf16 [p=(f8,k), col, fo],
    f = fo*8+f8, p = f8*16+k, 288 cols = 18 slots x 16 rows.  PSUM -> SBUF
    copies write fo-PAIRS (strided single-fo writes measured ~3x slow).
  * Per double unit: DVE bf16 tensor_sub (2x mode, fo-halved to overlap
    the GEMM) -> abs split between ACT (Abs, fp8 out -> DoubleRow k-sum)
    and DVE (sign-strip, bf16 -> plain bf16 k-sum in 64-row windows) ->
    k-sum on TensorE into pd[p=(i,f8), (j,fo)] -> ACT Exp(scale=-1) ->
    bf16 et, DMA'd out per unit.
  * Host: i-sums (over partition groups) and j-sums (over the free j axis)
    of each et block are scatter-added per the trail map; -1; concat x.
"""

import os
import numpy as np

import concourse.bass as bass
import concourse.tile as tile
from concourse import bacc, mybir

BF16 = mybir.dt.bfloat16
FP32 = mybir.dt.float32
FP8 = mybir.dt.float8e4
U16 = mybir.dt.uint16
NPBF16 = np.dtype(mybir.dt.np(BF16))
NPFP8 = np.dtype(mybir.dt.np(FP8))
DR = mybir.MatmulPerfMode.DoubleRow

B = 256
IN_F = 512
OUT_F = 128
NUM_K = 16
N_CORES = 8
STRIP = 16
NSLOT = 18
NDU = 8                    # double units; edge 16 handled as a single
NCOL = NSLOT * STRIP       # 288
F8 = 8
FO = OUT_F // F8           # 16

# abs split over the 16 i's of each (double) unit: N_ACT via ACT (Abs, fp8
# out -> DoubleRow k-sum, must be even), rest via DVE sign-strip (bf16 ->
# plain k-sum).
N_ACT = int(os.environ.get("N_ACT", "10"))
N_DVE = 16 - N_ACT
assert N_ACT % 2 == 0 and N_ACT > 0

SLOTS = [
    [0, 0, 1, 1, 15, 2, 14, 3, 13, 4, 12, 5, 11, 6, 10, 7, 9, 8],
    [1, 2, 2, 0, 3, 3, 15, 4, 14, 5, 13, 6, 12, 7, 11, 8, 10, 9],
    [2, 3, 1, 4, 4, 0, 5, 5, 15, 6, 14, 7, 13, 8, 12, 9, 11, 10],
    [3, 4, 2, 5, 1, 6, 6, 0, 7, 7, 15, 8, 14, 9, 13, 10, 12, 11],
    [4, 5, 3, 6, 2, 7, 1, 8, 8, 0, 9, 9, 15, 10, 14, 11, 13, 12],
    [5, 6, 4, 7, 3, 8, 2, 9, 1, 10, 10, 0, 11, 11, 15, 12, 14, 13],
    [6, 7, 5, 8, 4, 9, 3, 10, 2, 11, 1, 12, 12, 0, 13, 13, 15, 14],
    [7, 8, 6, 9, 5, 10, 4, 11, 3, 12, 2, 13, 1, 14, 14, 0, 15, 15],
]


def build_nc():
    nc = bacc.Bacc(name="minibatch_discrim_sym")

    xT_d = nc.dram_tensor("xT8", [128, 2, 2, NCOL], FP8, kind="ExternalInput")
    T_d = nc.dram_tensor("T8", [FO, 128, 2, 2, 128], FP8, kind="ExternalInput")
    # DoubleRow k-sum weights: ones8dr[v, p, t, r] = 1 iff r == 16*v + t*8 + p//16
    on8_d = nc.dram_tensor("ones8dr", [8, 128, 2 * 128], FP8, kind="ExternalInput")
    # plain k-sum weights: ones16[q, p, r] = 1 iff r == q*8 + p//16 (64-row windows)
    on16_d = nc.dram_tensor("ones16", [8, 128, 64], BF16, kind="ExternalInput")
    # per-unit exp blocks; host reduces them
    et_d = nc.dram_tensor("et_out", [NDU + 1, 128, 2 * STRIP * FO], BF16,
                          kind="ExternalOutput")

    with tile.TileContext(nc) as tc:
        with (
            tc.tile_pool(name="const", bufs=1) as constp,
            tc.tile_pool(name="mm", bufs=1) as mmp,
            tc.tile_pool(name="gpsum", bufs=2, space=bass.MemorySpace.PSUM) as gps,
            tc.tile_pool(name="dpsum", bufs=4, space=bass.MemorySpace.PSUM) as dps,
            tc.tile_pool(name="work", bufs=6) as wp,
            tc.tile_pool(name="expp", bufs=6) as ep,
        ):
            # ---- constants / inputs to SBUF ----
            zero_b = constp.tile([128, 1], FP32)
            nc.gpsimd.memset(zero_b[:], 0.0)

            ones8 = constp.tile([128, 8, 2, 128], FP8)
            nc.sync.dma_start(ones8[:], on8_d.rearrange("v p (t r) -> p v t r", r=128))
            ones16 = constp.tile([128, 8, 64], BF16)
            nc.sync.dma_start(ones16[:], on16_d.rearrange("q p r -> p q r"))

            # warm the ACT table while DMAs run
            warm = constp.tile([128, 1], FP32)
            nc.scalar.activation(
                warm[:], zero_b[:], mybir.ActivationFunctionType.Abs, bias=zero_b[:]
            )
            nc.scalar.activation(
                warm[:], zero_b[:], mybir.ActivationFunctionType.Exp, bias=zero_b[:]
            )

            xT_sb = constp.tile([128, 2, 2, NCOL], FP8)
            nc.sync.dma_start(xT_sb[:], xT_d[:])
            T_tiles = []
            for fo in range(FO):
                tt = constp.tile([128, 2, 2, 128], FP8, tag=f"T{fo}")
                nc.sync.dma_start(tt[:], T_d[fo])
                T_tiles.append(tt)

            # ---- GEMM (fp8 DoubleRow): m[p=(f8,k), col, fo] bf16 ----
            # two fo's accumulate into one 2-bank PSUM tile; the copy then
            # writes fo-pairs (4B bursts instead of 2B into the strided
            # [col, fo] layout), alternating ACT/DVE
            m_sb = mmp.tile([128, NCOL, FO], BF16)
            for g in range(FO // 2):
                pm2 = gps.tile([128, 2, 512], FP32, tag="gemm")
                for q in range(2):
                    for c2 in range(2):
                        nc.tensor.matmul(
                            pm2[:, q, :NCOL],
                            T_tiles[2 * g + q][:, c2],
                            xT_sb[:, c2],
                            start=(c2 == 0),
                            stop=(c2 == 1),
                            perf_mode=DR,
                        )
                src = pm2[:, :, :NCOL].rearrange("p q c -> p c q")
                if g % 2 == 0:
                    nc.scalar.copy(m_sb[:, :, 2 * g:2 * g + 2], src)
                else:
                    nc.vector.tensor_copy(m_sb[:, :, 2 * g:2 * g + 2], src)

            # ---- unit loop: 8 double units + 1 single ----
            def do_unit(d, jw, icol, jcol):
                diff = wp.tile([128, STRIP, jw, FO], BF16, tag="diff")
                for fh in range(2):
                    nc.vector.tensor_sub(
                        diff[:, :, :, 8 * fh:8 * (fh + 1)],
                        m_sb[:, icol:icol + STRIP, None, 8 * fh:8 * (fh + 1)]
                        .broadcast_to([128, STRIP, jw, 8]),
                        m_sb[:, None, jcol:jcol + jw, 8 * fh:8 * (fh + 1)]
                        .broadcast_to([128, STRIP, jw, 8]),
                    )

                ad8 = wp.tile([128, N_ACT, jw, FO], FP8, tag="ad8")
                nc.scalar.activation(
                    ad8[:], diff[:, :N_ACT],
                    mybir.ActivationFunctionType.Abs, bias=zero_b[:],
                )
                if N_DVE > 0:
                    ad16 = wp.tile([128, N_DVE, jw, FO], BF16, tag="ad16")
                    nc.vector.tensor_scalar(
                        ad16[:].bitcast(U16),
                        diff[:, N_ACT:].bitcast(U16),
                        0x7FFF, None, op0=mybir.AluOpType.bitwise_and,
                    )

                # k-sum into pd[p=(i,f8), (j,fo)]; DoubleRow must target dst
                # partition 0 (full-width weights), bf16 in 64-row windows
                pd = dps.tile([128, jw, FO], FP32, tag="dist")
                n_ops = N_ACT // 2 + N_DVE
                k_ = 0
                for p2 in range(N_ACT // 2):
                    nc.tensor.matmul(
                        pd[:], ones8[:, p2], ad8[:, 2 * p2:2 * p2 + 2],
                        start=(k_ == 0), stop=(k_ == n_ops - 1),
                        perf_mode=DR, tile_position=(0, 0),
                        skip_group_check=True,
                    )
                    k_ += 1
                for i0 in range(N_ACT, 16):
                    w64 = i0 // 8
                    nc.tensor.matmul(
                        pd[64 * w64:64 * (w64 + 1)],
                        ones16[:, i0 % 8], ad16[:, i0 - N_ACT],
                        start=(k_ == 0), stop=(k_ == n_ops - 1),
                        tile_position=(0, 64 * w64),
                        skip_group_check=True,
                    )
                    k_ += 1

                et = ep.tile([128, jw, FO], BF16, tag="expt")
                nc.scalar.activation(
                    et[:], pd[:],
                    mybir.ActivationFunctionType.Exp, bias=zero_b[:], scale=-1.0,
                )
                nc.sync.dma_start(
                    et_d[d, :, :jw * FO],
                    et[:].rearrange("p a b -> p (a b)"),
                )

            # m columns are ordered [s0,s2,..,s16, s1,s3,..,s17]: du d has
            # its two j-slots adjacent at col 16d, its i-slot at 144+16d
            for d in range(NDU):
                do_unit(d, 32, (9 + d) * STRIP, d * STRIP)
            do_unit(8, 16, 17 * STRIP, 8 * STRIP)

    nc.finalize()
    return nc


def make_in_maps(x: np.ndarray, T: np.ndarray):
    x8 = x.astype(NPFP8)
    T8f = T.astype(NPFP8)
    T8 = np.ascontiguousarray(
        T8f.reshape(2, 2, 128, FO, 128).transpose(3, 2, 0, 1, 4)
    )

    p = np.arange(128)
    t_ = np.arange(2)
    r128 = np.arange(128)
    on8 = (r128[None, None, None, :]
           == 16 * np.arange(8)[:, None, None, None]
           + 8 * t_[None, None, :, None]
           + (p[None, :, None, None] // 16)).astype(NPFP8)
    on8 = np.ascontiguousarray(on8.reshape(8, 128, 256))
    r64 = np.arange(64)
    on16 = np.ascontiguousarray(
        (r64[None, None, :] == 8 * np.arange(8)[:, None, None]
         + (p[None, :, None] // 16)).astype(NPBF16))

    in_maps = []
    for c in range(N_CORES):
        order = SLOTS[c][0::2] + SLOTS[c][1::2]
        rows = np.concatenate(
            [np.arange(s * STRIP, (s + 1) * STRIP) for s in order]
        )
        xp = x8[rows]
        xT8 = np.ascontiguousarray(
            xp.T.reshape(2, 2, 128, NCOL).transpose(2, 0, 1, 3)
        )
        in_maps.append({
            "xT8": xT8,
            "T8": T8,
            "ones8dr": on8,
            "ones16": on16,
        })
    return in_maps


def assemble(x: np.ndarray, results) -> np.ndarray:
    out_pair = np.zeros((B, OUT_F), np.float32)
    for c in range(N_CORES):
        s = SLOTS[c]
        et = results[c]["et_out"].astype(np.float32)   # [9, 128, 1024]
        for d in range(9):
            if d < 8:
                si, js = s[2 * d + 1], [s[2 * d], s[2 * d + 2]]
            else:
                si, js = s[17], [s[16]]
            nj = len(js)
            # e[(i,f8), (j2,j,fo)] -> [i, f8, j2, j, fo]
            e = et[d, :, :nj * STRIP * FO].reshape(STRIP, F8, nj, STRIP, FO)
            bi = e.sum(axis=0)            # [f8, j2, j, fo]
            bj = e.sum(axis=3)            # [i, f8, j2, fo]
            for h2, t in enumerate(js):
                out_pair[t * STRIP:(t + 1) * STRIP] += (
                    bi[:, h2].transpose(1, 2, 0).reshape(STRIP, OUT_F)
                )
                if t != si:
                    out_pair[si * STRIP:(si + 1) * STRIP] += (
                        bj[:, :, h2].transpose(0, 2, 1).reshape(STRIP, OUT_F)
                    )
    out_pair -= 1.0
    out = np.empty((B, IN_F + OUT_F), np.float32)
    out[:, :IN_F] = x
    out[:, IN_F:] = out_pair
    return out


_NC_CACHE = None


def kernel(x: np.ndarray, T: np.ndarray) -> np.ndarray:
    global _NC_CACHE
    from concourse import bass_utils

    if _NC_CACHE is None:
        _NC_CACHE = build_nc()
    nc = _NC_CACHE
    in_maps = make_in_maps(np.asarray(x, np.float32), np.asarray(T, np.float32))
    res = bass_utils.run_bass_kernel_spmd(nc, in_maps, core_ids=list(range(N_CORES)))
    return assemble(np.asarray(x, np.float32), res.results)
